# revision 21
# baseline (speedup 1.0000x reference)
"""BinaryLinear (4,2048,4096)x(4096,4096) on 8 TRN2 NeuronCores.

y = x @ (scale * sign(w)).T with scale = mean(|w|, axis=1).

Strategy: data-parallel over the 8192 flattened rows of x (1024 rows/core),
weight replicated. Per core:
  - x^T shard is DMA'd with an on-the-fly cast to bf16 and cached in SBUF.
  - w^T streams through SBUF in [128k x 512n] fp32 tiles; ACT computes
    sign(w)->bf16 tiles (exact +-1), DVE computes |w| and accumulates the
    per-column sums in fp32; a single fp32 matmul with a (1/4096)-constant
    stationary operand reduces the partition dim, broadcasting mean(|w|)
    to every PSUM partition.
  - Main compute: 2048 bf16 matmuls (lhsT = x^T k,m-tile, rhs = sign tile)
    accumulating over k into PSUM; the PSUM->SBUF copy is fused with the
    per-column scale multiply on DVE; fp32 results DMA out.
The sign matrix is exact in bf16, so the only precision loss vs the fp32
reference is the bf16 rounding of x (~1e-3 relative).
"""

import sys

for _p in ("/opt/trn_rl_repo",):
    if _p not in sys.path:
        sys.path.append(_p)

import numpy as np

import concourse.bass as bass
import concourse.mybir as mybir
import concourse.tile as tile
from concourse import bacc
from concourse.bass_isa import ReduceOp
from concourse.bass_utils import run_bass_kernel_spmd

P = 128
K_DIM = 4096          # contraction (in_chn)
KT = K_DIM // P       # 32 k-tiles
N_DIM = 4096          # out_chn
NT = 512              # n tile (PSUM bank width in fp32)
N_TILES = N_DIM // NT
N_CORES = 8
M_FULL = 4 * 2048     # flattened batch rows
M_LOC = M_FULL // N_CORES
MT = M_LOC // P

f32 = mybir.dt.float32
bf16 = mybir.dt.bfloat16


def build_kernel(
    repeat: int = 1,
    # ablation switches for TimelineSim analysis only (defaults = real kernel)
    no_x: bool = False,
    no_scale: bool = False,
    no_wprep: bool = False,
    # perf variants (defaults = current best)
    offload_dve: bool = False,  # abs/acc on GpSimd + scale copy on ACT (slower)
    nt0_kouter: bool = True,    # k-outer MM groups for nt=0 (startup overlap)
    x_hwdge: bool = False,      # load x via HWDGE + DVE cast (no SWDGE)
    timing_mode: bool = False,  # out DMAs -> internal DRAM; tiny ext output
    nt0_wide: bool = True,      # nt=0 k-outer covers all 8 mt (7+1 psum)
    swdge_queues: int = 2,
    x_bf16_host: bool = True,   # x arrives bf16 (host-cast); halves x DMA
    out_on_act: bool = True,    # out DMAs on ACT HWDGE ring (SP ring = w only)
    w_slab: int = 4,            # k-tiles per w stage DMA
    x_msplit: bool = False,     # load x in m-halves; nt0 groups consume halves
    small_first_slabs: bool = True,  # 1-k-tile first w/x DMAs (faster start)
    scale_on_pool: bool = True,  # scale reduce via GpSimd, not PE ones-matmul
    unroll: int = 1,            # bodies per For_i iteration (pool-cycled)
    x_split: int = 8,           # k-tiles in double-buffered x prefix (0=off)
    x_on_act: bool = False,     # x DMAs on ACT HWDGE ring (needs bf16 host x)
):
    nc = bacc.Bacc(
        "TRN2", target_bir_lowering=False, num_swdge_queues=swdge_queues
    )
    xt = nc.dram_tensor(
        "xt", [K_DIM, M_LOC], bf16 if x_bf16_host else f32,
        kind="ExternalInput",
    )
    wt = nc.dram_tensor("wt", [K_DIM, N_DIM], f32, kind="ExternalInput")
    if timing_mode:
        y = nc.dram_tensor("y", [P, 16], f32, kind="ExternalOutput")
        y_scr = nc.dram_tensor("y_scr", [M_LOC, N_DIM], f32)
        y_r = y_scr.rearrange("(mt p) n -> p mt n", p=P)
    else:
        y = nc.dram_tensor("y", [M_LOC, N_DIM], f32, kind="ExternalOutput")
        y_r = y.rearrange("(mt p) n -> p mt n", p=P)

    xt_r = xt.rearrange("(kt p) m -> p kt m", p=P)
    wt_r = wt.rearrange("(kt p) n -> p kt n", p=P)

    with tile.TileContext(nc) as tc:
        with (
            tc.tile_pool(name="xcache", bufs=1) as xcache_pool,
            tc.tile_pool(name="const", bufs=1) as const_pool,
            tc.tile_pool(name="xstage", bufs=2) as xstage_pool,
            tc.tile_pool(name="xa", bufs=2) as xa_pool,
            tc.tile_pool(name="wstage", bufs=2 if x_hwdge else 3) as wstage_pool,
            tc.tile_pool(name="absw", bufs=3) as absw_pool,
            tc.tile_pool(name="sgn", bufs=2) as sgn_pool,
            tc.tile_pool(name="acc", bufs=2) as acc_pool,
            tc.tile_pool(name="scale", bufs=2) as scale_pool,
            tc.tile_pool(name="out", bufs=4) as out_pool,
            tc.tile_pool(
                name="psum_s", bufs=1 if nt0_wide else 2, space="PSUM"
            ) as psum_s_pool,
            tc.tile_pool(
                name="psum_y",
                bufs=(8 if scale_on_pool else 7) if nt0_wide else 4,
                space="PSUM",
            ) as psum_y_pool,
        ):
            if not scale_on_pool:
                ones = const_pool.tile([P, P], f32, tag="ones")
                nc.vector.memset(ones[:], 1.0 / K_DIM)
            use_xsplit = x_split > 0 and not (no_x or x_hwdge or x_msplit)
            KA = x_split
            if not use_xsplit:
                xcache = xcache_pool.tile([P, KT, M_LOC], bf16, tag="xc")

            def body(_i=None):
                if use_xsplit:
                    # k-prefix double-buffered: next body's xa prefetch
                    # overlaps this body's tail, so nt0's first MMs never
                    # wait on the x preload at iteration boundaries.
                    xa = xa_pool.tile([P, KA, M_LOC], bf16, tag="xa")
                    xb = xcache_pool.tile([P, KT - KA, M_LOC], bf16, tag="xb")

                    def xsl(k, msl):
                        if k < KA:
                            return xa[:, k, msl]
                        return xb[:, k - KA, msl]

                else:

                    def xsl(k, msl):
                        return xcache[:, k, msl]

                # Load + cast x^T shard to bf16 (SWDGE casts in-flight).
                if no_x:
                    nc.gpsimd.memset(xcache[:, :, 0:8], 1.0)
                elif x_hwdge:
                    # x on ACT's HWDGE ring (separate FIFO from the w stream
                    # on SP's ring), cast f32->bf16 on DVE.
                    for c in range(0, KT, 2):
                        xstage = xstage_pool.tile(
                            [P, 2, M_LOC], f32, tag="xs", name="xs"
                        )
                        nc.scalar.dma_start(xstage[:], xt_r[:, c : c + 2, :])
                        nc.vector.tensor_copy(
                            xcache[:, c : c + 2, :], xstage[:]
                        )
                elif x_msplit:
                    # m-halves: nt0 group A (mt 0-3) only needs half the x
                    # bytes before it can run at full MM pace.
                    mh = M_LOC // 2
                    for h in range(2):
                        msl = bass.ds(h * mh, mh)
                        for c in range(0, KT, 8):
                            nc.gpsimd.dma_start(
                                xcache[:, c : c + 8, msl],
                                xt_r[:, c : c + 8, msl],
                            )
                else:
                    if small_first_slabs:
                        xslabs = [(0, 1), (1, 1), (2, 2)] + [
                            (c, 4) for c in range(4, KT, 4)
                        ]
                    else:
                        xslabs = [(c, 4) for c in range(0, KT, 4)]
                    for c, sl in xslabs:
                        if use_xsplit:
                            assert c + sl <= KA or c >= KA
                            dst = (
                                xa[:, c : c + sl, :]
                                if c < KA
                                else xb[:, c - KA : c - KA + sl, :]
                            )
                        else:
                            dst = xcache[:, c : c + sl, :]
                        x_eng = nc.scalar if x_on_act else nc.gpsimd
                        x_eng.dma_start(dst, xt_r[:, c : c + sl, :])

                for nt_i in range(N_TILES):
                    nsl = bass.ts(nt_i, NT)
                    sgn = sgn_pool.tile([P, KT, NT], bf16, tag="sgn")
                    acc = acc_pool.tile([P, NT], f32, tag="acc")
                    if no_wprep:
                        nc.gpsimd.memset(sgn[:, :, 0:8], 1.0)
                    if not no_wprep:
                        if small_first_slabs and nt_i == 0 and w_slab == 4:
                            wslabs = [(0, 1), (1, 1), (2, 2)] + [
                                (kc, w_slab) for kc in range(4, KT, w_slab)
                            ]
                        else:
                            wslabs = [
                                (kc, w_slab) for kc in range(0, KT, w_slab)
                            ]
                        for kc, wsl in wslabs:
                            wstage = wstage_pool.tile(
                                [P, w_slab, NT], f32, tag="ws"
                            )
                            nc.sync.dma_start(
                                wstage[:, 0:wsl, :],
                                wt_r[:, kc : kc + wsl, nsl],
                            )
                            for j in range(wsl):
                                k = kc + j
                                nc.scalar.sign(sgn[:, k, :], wstage[:, j, :])
                                if no_scale:
                                    continue
                                # |w| exactly via sign-bit clear on DVE
                                # (abs_max is not a valid TRN2 tensor op).
                                if k == 0:
                                    abs_dst = acc[:]
                                else:
                                    absw = absw_pool.tile(
                                        [P, NT], f32, tag="absw", name="absw"
                                    )
                                    abs_dst = absw[:]
                                # abs/accumulate off the critical DVE so DVE
                                # only drains PSUM (keeps PE bank recycling
                                # prompt); GpSimd is otherwise idle.
                                eng = nc.gpsimd if offload_dve else nc.vector
                                eng.tensor_scalar(
                                    abs_dst.bitcast(mybir.dt.uint32),
                                    wstage[:, j, :].bitcast(mybir.dt.uint32),
                                    0x7FFFFFFF, None,
                                    mybir.AluOpType.bitwise_and,
                                )
                                if k > 0:
                                    eng.tensor_add(acc[:], acc[:], abs_dst)
                    scale_sb = scale_pool.tile([P, NT], f32, tag="scale_sb")
                    if not (no_scale or no_wprep):
                        if scale_on_pool:
                            # All-reduce acc over partitions on GpSimd (result
                            # lands broadcast on every partition); scale by
                            # 1/K for the mean. PE stays on the main matmuls.
                            nc.gpsimd.partition_all_reduce(
                                scale_sb[:], acc[:], P, ReduceOp.add
                            )
                            nc.gpsimd.tensor_scalar(
                                scale_sb[:], scale_sb[:], 1.0 / K_DIM, None,
                                mybir.AluOpType.mult,
                            )
                        else:
                            # Reduce acc over partitions (fp32 matmul with 1/K
                            # ones); every PSUM partition receives the same
                            # column sums = mean(|w|) broadcast.
                            scale_ps = psum_s_pool.tile(
                                [P, NT], f32, tag="scale_ps"
                            )
                            nc.tensor.matmul(
                                scale_ps[:], lhsT=ones[:], rhs=acc[:],
                                start=True, stop=True,
                            )
                            if offload_dve:
                                nc.scalar.copy(scale_sb[:], scale_ps[:])
                            else:
                                nc.vector.tensor_copy(
                                    scale_sb[:], scale_ps[:]
                                )

                    def emit_out(y_ps, mt_i):
                        out_sb = out_pool.tile([P, NT], f32, tag="out_sb")
                        if no_scale or no_wprep:
                            nc.vector.tensor_copy(out_sb[:], y_ps[:])
                        else:
                            nc.vector.tensor_tensor(
                                out_sb[:], y_ps[:], scale_sb[:],
                                mybir.AluOpType.mult,
                            )
                        out_eng = nc.scalar if out_on_act else nc.sync
                        out_eng.dma_start(y_r[:, mt_i, nsl], out_sb[:])
                        if timing_mode and nt_i == 0 and mt_i == 0:
                            out_eng.dma_start(y[:], out_sb[:, 0:16])

                    if nt0_kouter and nt_i == 0:
                        # First n-tile: k-outer over wide mt groups so the
                        # PE consumes x/sgn tiles as their DMAs land instead
                        # of stalling for the full x preload.
                        gw = MT if nt0_wide else 4
                        for mg in range(0, MT, gw):
                            group = list(range(mg, mg + gw))
                            pss = {}
                            for mt_i in group:
                                pss[mt_i] = psum_y_pool.tile(
                                    [P, NT], f32, tag="y_ps", name="y_ps"
                                )
                            for k in range(KT):
                                for mt_i in group:
                                    nc.tensor.matmul(
                                        pss[mt_i][:],
                                        lhsT=xsl(k, bass.ts(mt_i, P)),
                                        rhs=sgn[:, k, :],
                                        start=(k == 0),
                                        stop=(k == KT - 1),
                                    )
                            for mt_i in group:
                                emit_out(pss[mt_i], mt_i)
                    else:
                        for mt_i in range(MT):
                            y_ps = psum_y_pool.tile(
                                [P, NT], f32, tag="y_ps", name="y_ps"
                            )
                            for k in range(KT):
                                nc.tensor.matmul(
                                    y_ps[:],
                                    lhsT=xsl(k, bass.ts(mt_i, P)),
                                    rhs=sgn[:, k, :],
                                    start=(k == 0),
                                    stop=(k == KT - 1),
                                )
                            emit_out(y_ps, mt_i)

            eff_unroll = unroll if repeat % unroll == 0 else 1
            n_iters = repeat // eff_unroll

            def bodies():
                for _u in range(eff_unroll):
                    body()

            if n_iters == 1:
                bodies()
            else:
                with tc.For_i(0, n_iters, 1) as _i:
                    bodies()

    nc.compile()
    return nc


def _shard_inputs(x: np.ndarray, weight: np.ndarray, x_bf16_host=True):
    xt = x.reshape(M_FULL, K_DIM).T  # [K, M_FULL] view
    if x_bf16_host:
        import ml_dtypes

        xt = xt.astype(ml_dtypes.bfloat16)
    wt = np.ascontiguousarray(weight.T)  # [K, N]
    in_maps = []
    for c in range(N_CORES):
        shard = np.ascontiguousarray(xt[:, c * M_LOC : (c + 1) * M_LOC])
        in_maps.append({"xt": shard, "wt": wt})
    return in_maps


def kernel(x: np.ndarray, weight: np.ndarray) -> np.ndarray:
    x = np.asarray(x, dtype=np.float32)
    weight = np.asarray(weight, dtype=np.float32)
    nc = build_kernel(repeat=1)
    in_maps = _shard_inputs(x, weight)
    res = run_bass_kernel_spmd(nc, in_maps, core_ids=list(range(N_CORES)))
    y = np.concatenate([res.results[c]["y"] for c in range(N_CORES)], axis=0)
    return y.reshape(x.shape[0], x.shape[1], N_DIM).astype(np.float32)

